# revision 1
# baseline (speedup 1.0000x reference)
"""Trainium2 Bass kernel for nn_LSTMMachine: tiny LSTM (H=2) over B=8192, T=64, I=125.

Strategy (pure data parallelism, hint-aligned):
  - Shard batch B across 8 cores (1024 rows each); replicate the tiny weights.
  - Host-side shard prep lays each core's image slice out as x2[feature, (t, q, b)]
    in fp16 so the PE can consume it directly as matmul stationary tiles
    (contraction dim = feature on partitions). A constant `ones` row folds the
    bias add into the same matmul. fp16 halves HBM traffic and PE weight-load
    time while keeping ~8x more mantissa than bf16 (bf16 noise through the
    64-step recurrence measured right at the 2e-2 tolerance; fp16 is ~4e-3).
  - Per core/per timestep t: 8 matmuls (stationary = x-chunk [126,128] fp16,
    moving = W_ih^T [126,8] fp16) produce gate pre-activations for all 1024
    local rows in scan layout [128 part = batch, (q,r)] in PSUM.
  - The sequential scan is latency-bound: the per-step ring is
      sigma1(8 gate rows, one ACT op; g-rows pre-scaled x2 so
      tanh(x) = 2*sigmoid(2x)-1) -> mb (DVE) [ma = sig_f*c on Pool in
      parallel] -> C (DVE) -> sigma2 = sigmoid(2*C) (ACT, scale fused) ->
      u = (S2-0.5)*vo (one DVE STT; state is (i,q)-major so both hidden
      channels flatten into one 3-dim AP) -> s -> g (DVE) -> sigma1 ...
    h is never materialized: h = 2*sig_o*(sig(2c)-0.5) folds into
    vo = sig_o (x) 2*W_hh (computed on Pool, off the ring).
  - Two half-phase software-pipelined chains (4 q-groups each): engines are
    in-order with head-of-line blocking, so emission order is the schedule;
    chain B trails chain A by half a step and every engine stream is emitted
    in true firing order. Chain A reads gates_x straight from PSUM; chain
    B's half is staged to SBUF by an ACT copy in the sigma slack.
"""
import sys

sys.path.insert(0, "/opt/trn_rl_repo")

import numpy as np

B, T, I, H = 8192, 64, 125, 2
NCORES = 8
P = 128
BL = B // NCORES  # 1024
Q = BL // P  # 8
R = 4 * H  # 8 gate rows
# torch gate order [i0 i1 f0 f1 g0 g1 o0 o1] -> ours [f0 f1 i0 i1 o0 o1 g0 g1]
PERM = [2, 3, 0, 1, 6, 7, 4, 5]

_CACHE = {}


def build_nc(q=Q, t_steps=T, i_feat=I, num_devices=NCORES, repeats=1, nchains=2):
    """Build the per-core Bass module. Generic in (q, t_steps, i_feat) so a
    small configuration can be validated cheaply. repeats>1 statically
    replays the whole pipeline (timing experiments only). nchains splits the
    local batch into independent scan chains to hide serial-chain latency."""
    import concourse.bacc as bacc
    import concourse.mybir as mybir
    import concourse.tile as tile

    f32 = mybir.dt.float32
    f16 = mybir.dt.float16
    Alu = mybir.AluOpType
    Act = mybir.ActivationFunctionType

    kf = i_feat + 1  # +1 ones row carries the bias
    ncols = t_steps * q * P
    w_g = q * R  # free width of one timestep's gates
    nc = bacc.Bacc("TRN2", target_bir_lowering=False, debug=False,
                   num_devices=num_devices)

    x2 = nc.dram_tensor("x2", [kf, ncols], f16, kind="ExternalInput")
    wih = nc.dram_tensor("wih", [kf, R], f16, kind="ExternalInput")
    whh = nc.dram_tensor("whh", [P, 2 * w_g], f32, kind="ExternalInput")
    fcw = nc.dram_tensor("fcw", [P, 2 * q], f32, kind="ExternalInput")
    fcb = nc.dram_tensor("fcb", [P, 1], f32, kind="ExternalInput")
    out = nc.dram_tensor("out", [P, q], f32, kind="ExternalOutput")

    # DMA chunking: group timesteps into ~1MB loads
    per_t_bytes = kf * q * P * 2
    chunk_t = max(1, int(np.ceil(258_000 / per_t_bytes)))
    chunk_t = min(chunk_t, t_steps)
    while t_steps % chunk_t:
        chunk_t -= 1
    tcols = q * P

    with tile.TileContext(nc) as tc:
        with (
            tc.tile_pool(name="consts", bufs=1) as consts,
            tc.tile_pool(name="state", bufs=1) as state,
            tc.tile_pool(name="xin", bufs=8) as xin,
            tc.tile_pool(name="psum", bufs=6, space="PSUM") as psum,
            tc.tile_pool(name="gx", bufs=3) as gxp,
            tc.tile_pool(name="sg", bufs=3) as sgp,
            tc.tile_pool(name="gg", bufs=3) as ggp,
            tc.tile_pool(name="vo", bufs=3) as vop,
            tc.tile_pool(name="mm", bufs=3) as mmp,
        ):
            wih_t = consts.tile([kf, R], f16)
            nc.sync.dma_start(wih_t[:], wih[:])
            whh_t = consts.tile([P, 2 * w_g], f32)
            nc.sync.dma_start(whh_t[:], whh[:])
            fcw_t = consts.tile([P, 2 * q], f32)
            nc.sync.dma_start(fcw_t[:], fcw[:])
            fcb_t = consts.tile([P, 1], f32)
            nc.sync.dma_start(fcb_t[:], fcb[:])

            assert q % nchains == 0
            qc = q // nchains  # q-groups per chain
            wc = qc * R        # gate free-width per chain
            # state layout is (i, q)-major: m = i*qc + q matches vo's (i,q,r)
            # halves, so one 3-dim STT covers both hidden channels
            Cs, S2s = [], []
            for ch in range(nchains):
                Cch = state.tile([P, 2 * qc], f32, tag=f"C{ch}")   # c, (i, q)
                S2ch = state.tile([P, 2 * qc], f32, tag=f"S2{ch}")  # sig(2c), (i, q)
                nc.vector.memset(Cch[:], 0.0)
                nc.vector.memset(S2ch[:], 0.5)
                Cs.append(Cch)
                S2s.append(S2ch)

            # touch the sigmoid table set immediately so its ~1.3us load
            # overlaps the first DMA chunk instead of the first scan step
            warm = state.tile([P, 1], f32, tag="warm")
            nc.scalar.activation(warm[:], Cs[0][:, 0:1], Act.Sigmoid)

            whh_v = whh_t[:].rearrange("p (i q r) -> p i q r", i=2, q=q)

            # per-chain rolling refs to the last vo tile
            vos = [None] * nchains

            sgs = [None] * nchains

            def scan_h1(ch, gx_sb, first):
                """gate preactivations + sigma1 for (ch, t)."""
                S2 = S2s[ch]
                g = ggp.tile([P, wc], f32, tag=f"g{ch}")
                if first:
                    # h=0 at t=0: g = gx
                    nc.vector.tensor_scalar_add(g[:], gx_sb, 0.0)
                else:
                    # u = (sig(2c) - 0.5) * vo in ONE 3-dim STT: the (i,q)
                    # state layout flattens to m = i*qc+q with uniform stride
                    vo_prev = vos[ch][:]
                    up = mmp.tile([P, 2 * wc], f32, tag=f"up{ch}")
                    s2b = S2[:].broadcast_to((P, 2 * qc, R))
                    nc.vector.scalar_tensor_tensor(
                        up[:].rearrange("p (m r) -> p m r", r=R),
                        s2b, 0.5,
                        vo_prev.rearrange("p (m r) -> p m r", r=R),
                        Alu.subtract, Alu.mult,
                    )
                    ss = ggp.tile([P, wc], f32, tag=f"s{ch}")
                    nc.vector.tensor_tensor(
                        ss[:], up[:, 0:wc], up[:, wc:2 * wc], Alu.add)
                    nc.vector.tensor_tensor(g[:], ss[:], gx_sb, Alu.add)
                sg = sgp.tile([P, wc], f32, tag=f"sg{ch}")
                nc.scalar.activation(sg[:], g[:], Act.Sigmoid)
                sgs[ch] = sg
                return sg

            def scan_h2(ch):
                """c-update + sigma2 + next-step vo for (ch, t)."""
                C, S2, sg = Cs[ch], S2s[ch], sgs[ch]
                sgr = sg[:].rearrange("p (q r) -> p r q", q=qc)
                # c update: C = 2*(sig2g-0.5)*sig_i + sig_f*C
                mb = mmp.tile([P, 2 * qc], f32, tag=f"mb{ch}")
                nc.vector.scalar_tensor_tensor(
                    mb[:].rearrange("p (i q) -> p i q", i=2),
                    sgr[:, 6:8, :], 0.5, sgr[:, 2:4, :],
                    Alu.subtract, Alu.mult,
                )
                # ma on Pool: off the DVE, runs parallel to mb
                ma = mmp.tile([P, 2 * qc], f32, tag=f"ma{ch}")
                nc.gpsimd.tensor_tensor(
                    ma[:].rearrange("p (i q) -> p i q", i=2),
                    sgr[:, 0:2, :],
                    C[:].rearrange("p (i q) -> p i q", i=2), Alu.mult)
                nc.vector.scalar_tensor_tensor(
                    C[:], mb[:], 2.0, ma[:], Alu.mult, Alu.add)
                # sigma2 = sigmoid(2c) (scale fused into ACT)
                nc.scalar.activation(S2[:], C[:], Act.Sigmoid, scale=2.0)
                # vo = sig_o (*) whh2, consumed by h1 of the NEXT step
                vo = vop.tile([P, 2 * wc], f32, tag=f"vo{ch}")
                ob = sg[:].rearrange("p (q r) -> p r q", q=qc)[:, 4:6, :]\
                    .broadcast_to((P, 2, qc, R))
                nc.gpsimd.tensor_tensor(
                    vo[:].rearrange("p (i q r) -> p i q r", i=2, q=qc),
                    ob, whh_v[:, :, ch * qc:(ch + 1) * qc, :], Alu.mult,
                )
                vos[ch] = vo
            # Software-pipelined 2-chain schedule: chain B runs half a step
            # behind chain A so each in-order engine stream (ACT especially)
            # is emitted in true firing order -- in-order engines suffer
            # head-of-line blocking, so emission order IS the schedule.
            # Chain A reads gates_x from PSUM; chain B's half is staged to
            # SBUF by an ACT copy in the slack after sigma1(A).
            assert nchains == 2
            A, Bc = 0, 1
            for rep_i in range(repeats):
              for tc_i in range(t_steps // chunk_t):
                xt = xin.tile([kf, chunk_t * tcols], f16)
                nc.sync.dma_start(
                    xt[:], x2[:, tc_i * chunk_t * tcols:(tc_i + 1) * chunk_t * tcols]
                )
                for dt_ in range(chunk_t):
                    t = tc_i * chunk_t + dt_
                    # full-bank tile: small PSUM tiles pack into one
                    # 2KB bank and serialize accumulation groups
                    ptf = psum.tile([P, 512], f32)
                    pt = ptf[:]
                    for qq in range(q):
                        nc.tensor.matmul(
                            pt[:, qq * R:(qq + 1) * R],
                            xt[:, dt_ * tcols + qq * P: dt_ * tcols + (qq + 1) * P],
                            wih_t[:],
                            start=True,
                            stop=True,
                        )
                    first = rep_i == 0 and t == 0
                    scan_h1(A, pt[:, 0:wc], first)
                    gx = gxp.tile([P, wc], f32)
                    nc.scalar.copy(gx[:], pt[:, wc:2 * wc])
                    if not first:
                        scan_h2(Bc)
                    scan_h2(A)
                    scan_h1(Bc, gx[:], first)
            scan_h2(Bc)
            sg_last = sgs

            # --- FC tail: h = 2*sig_o*(S2-0.5); out = h0*w0 + h1*w1 + b ---
            for ch in range(nchains):
                qs = slice(ch * qc, (ch + 1) * qc)
                hh = mmp.tile([P, 2 * qc], f32, tag=f"hh{ch}")
                sgr = sg_last[ch][:].rearrange("p (q r) -> p r q", q=qc)
                nc.vector.scalar_tensor_tensor(
                    hh[:].rearrange("p (i q) -> p i q", i=2),
                    S2s[ch][:].rearrange("p (i q) -> p i q", i=2), 0.5,
                    sgr[:, 4:6, :], Alu.subtract, Alu.mult,
                )
                mfc = mmp.tile([P, 2 * qc], f32, tag=f"mfc{ch}")
                # hh and fcw are both (i q)-major so res can slice i-halves
                nc.vector.tensor_tensor(
                    mfc[:].rearrange("p (i q) -> p i q", i=2),
                    hh[:].rearrange("p (i q) -> p i q", i=2),
                    fcw_t[:].rearrange("p (i q) -> p i q", i=2)[:, :, qs],
                    Alu.mult,
                )
                res = mmp.tile([P, qc], f32, tag=f"res{ch}")
                nc.vector.scalar_tensor_tensor(
                    res[:], mfc[:, 0:qc], fcb_t[:, 0:1], mfc[:, qc:2 * qc],
                    Alu.add, Alu.add,
                )
                nc.sync.dma_start(out[:, ch * qc:(ch + 1) * qc], res[:])

    nc.compile()
    return nc


def _to_f16(a):
    return np.asarray(a, dtype=np.float32).astype(np.float16)


def prep_core_inputs(image_c, W_ih, W_hh, b_ih, b_hh, fc_w, fc_b,
                     q=Q, t_steps=T, i_feat=I):
    """Host-side shard prep for one core's batch slice image_c [q*128, T, I]."""
    bl = q * P
    kf = i_feat + 1
    # x2[(feature), (t, q, b)]
    arr = np.asarray(image_c, np.float32).reshape(
        q, P, t_steps, i_feat).transpose(3, 2, 0, 1)
    x2 = np.empty((kf, t_steps * bl), dtype=np.float32)
    x2[:i_feat] = arr.reshape(i_feat, t_steps * bl)
    x2[i_feat] = 1.0

    bias = (b_ih + b_hh).astype(np.float32)
    # rows r=6,7 (the cell-candidate gate) are pre-scaled x2 so the kernel can
    # use a single sigmoid table lookup: tanh(x) = 2*sigmoid(2x) - 1.
    rscale = np.array([1.0, 1.0, 1.0, 1.0, 1.0, 1.0, 2.0, 2.0], np.float32)
    wihm = np.empty((kf, R), dtype=np.float32)
    for r in range(R):
        wihm[:i_feat, r] = W_ih[PERM[r]] * rscale[r]
        wihm[i_feat, r] = bias[PERM[r]] * rscale[r]

    # whh2[(i, q, r)] = 2 * W_hh[PERM[r], i] * rscale[r]
    # (the factor 2 folds h = 2*sig_o*(sig(2c)-0.5) into the constant)
    whh = np.empty((P, 2 * q * R), dtype=np.float32)
    for i in range(2):
        for qq in range(q):
            for r in range(R):
                whh[:, i * q * R + qq * R + r] = 2.0 * W_hh[PERM[r], i] * rscale[r]

    fcw = np.empty((P, 2 * q), dtype=np.float32)
    for i in range(2):
        # the factor 2 of h folds into fc_w here
        fcw[:, i * q:(i + 1) * q] = 2.0 * fc_w[0, i]
    fcb = np.full((P, 1), fc_b[0], dtype=np.float32)
    return {"x2": _to_f16(np.ascontiguousarray(x2)), "wih": _to_f16(wihm),
            "whh": whh, "fcw": fcw, "fcb": fcb}


def kernel(image, W_ih, W_hh, b_ih, b_hh, fc_w, fc_b):
    from concourse.bass_utils import run_bass_kernel_spmd

    image = np.asarray(image, dtype=np.float32)
    if "nc" not in _CACHE:
        _CACHE["nc"] = build_nc(nchains=2)
    nc = _CACHE["nc"]

    in_maps = []
    for c in range(NCORES):
        in_maps.append(
            prep_core_inputs(
                image[c * BL:(c + 1) * BL],
                np.asarray(W_ih, np.float32), np.asarray(W_hh, np.float32),
                np.asarray(b_ih, np.float32), np.asarray(b_hh, np.float32),
                np.asarray(fc_w, np.float32), np.asarray(fc_b, np.float32),
            )
        )
    res = run_bass_kernel_spmd(nc, in_maps, list(range(NCORES)))
    outp = np.empty((B, 1), dtype=np.float32)
    for c in range(NCORES):
        oc = res.results[c]["out"]  # [128, Q] -> b_local = q*128 + p
        outp[c * BL:(c + 1) * BL, 0] = oc.T.reshape(BL)
    return outp

